# revision 30
# baseline (speedup 1.0000x reference)
"""Single-head causal attention kernel for Trainium2 (8 NeuronCores).

Problem: x[8, 2048, 1024], Wq/Wk/Wv[1024, 64] ->
  out[b] = softmax(causal((x[b] @ Wq) @ (x[b] @ Wk)^T / 8)) @ (x[b] @ Wv)

Sharding: data-parallel over batch, one batch element per core, weights
replicated.

Design notes:
  - all matmul inputs bf16 (x/W converted on host): halves DMA bytes,
    full PE rate (1 cycle/row)
  - host prepacks x into [p, block, chunk, t] and W into [p, chunk, h];
    DMA order: wqk first, block-0 x in chunk pairs, wv, block-1 in
    halves, blocks 2-3 whole -- the first projection matmul starts as
    soon as (wqk, x-pair-0) land and then tracks the DMA engine
  - PE p-state handling: a tiny matmul right at start anchors the ramp
    clock, a wqk-gated matmul bridges the idle gap, and two x-gated
    matmuls hold the 4-deep PE wait queue so the first real matmuls are
    priced at data-arrival time (full 2.4GHz from the first instruction)
  - q/k projected packed ([Wq|Wk] stationary -> [128, t] psum, one pass
    for both); v projected directly in NATURAL [t, h] layout using the
    x chunk as stationary and Wv as moving (64-wide matmuls, no
    transposes)
  - boundary copies split q on DVE / k on gpsimd so they run in
    parallel; block 0 additionally splits them in column halves and
    emits its diagonal scores in [r2, r3, r0, r1] order for an earlier
    first exp
  - scores computed transposed: ST_j = kT_j^T . qT; OFF-DIAGONAL
    s-chunks are computed in PAIRS into a 2-bank [128,1024] psum tile
    and exp'd with ONE 1024-wide ACT instruction (halves the ~190ns
    per-instruction ACT overhead); causal diag tiles are masked
    additively with -1e9 before the exp, so the below-diagonal PT
    region is exactly 0
  - PV is computed TRANSPOSED, 128-t-columns at a time: out[t-part, 65]
    accumulates PT_j[:, g]^T . v_aug_j over s-chunks j (PT slice is the
    stationary operand, 65-wide moving v) -- 65 cycles per (chunk,
    group) instead of 512 per chunk, and the output lands in natural
    [t, h] layout with the softmax denominator (ones column of v_aug)
    as column 64: a per-PARTITION scalar
  - normalize is then reciprocal([128,1]) + tensor_scalar_mul on DVE (no
    partition-broadcast, no transposed output DMA); deferred into the
    next block's emission, off the critical path
  - emission interleaves att(b) with v(b) and qk(b+1) as PE fillers;
    block 3 runs six off-diagonal chunks early at the att(2) boundary
    (their exps fill the ACT seam) and orders its units pairs-first,
    diagonals-last so the four 128-row output groups close staggered,
    each normalize+DMA chain overlapping the next diag exp; the final
    group's DMA issues from the idle ACT queue
"""

import numpy as np
import ml_dtypes
from contextlib import ExitStack

import concourse.bass as bass
import concourse.tile as tile
import concourse.bacc as bacc
from concourse import mybir
from concourse import bass_utils

F32 = mybir.dt.float32
BF16 = mybir.dt.bfloat16

T = 2048
C = 1024
H = 64
NCH = C // 128   # 8 contraction chunks
NB = T // 512    # 4 t-blocks
NEG = -1.0e9
EXP = mybir.ActivationFunctionType.Exp


def build_bass():
    nc = bacc.Bacc("TRN2", target_bir_lowering=False, debug=False, num_devices=8)
    xp = nc.dram_tensor("xp", [128, NB, NCH, 512], BF16, kind="ExternalInput").ap()
    wqk = nc.dram_tensor("wqk", [128, NCH, 128], BF16, kind="ExternalInput").ap()
    wv = nc.dram_tensor("wv", [128, NCH, 64], BF16, kind="ExternalInput").ap()
    # natural [t, h] output, tiled [block, group, partition, h]
    outN = nc.dram_tensor("outN", [NB, 4, 128, H], F32, kind="ExternalOutput").ap()

    with tile.TileContext(nc) as tc:
        with ExitStack() as ctx:
            build_kernel(ctx, tc, nc, xp, wqk, wv, outN)
    nc.compile()
    return nc


def build_kernel(ctx, tc, nc, xp, wqk, wv, outN):
    const = ctx.enter_context(tc.tile_pool(name="const", bufs=1))
    pt_pool = ctx.enter_context(tc.tile_pool(name="pt", bufs=1))
    fin_pool = ctx.enter_context(tc.tile_pool(name="fin", bufs=2))
    rc_pool = ctx.enter_context(tc.tile_pool(name="rc", bufs=8))
    qk_ps = ctx.enter_context(tc.tile_pool(name="qkps", bufs=1, space="PSUM"))
    v_ps = ctx.enter_context(tc.tile_pool(name="vps", bufs=1, space="PSUM"))
    st_ps = ctx.enter_context(tc.tile_pool(name="stps", bufs=2, space="PSUM"))
    o_ps = ctx.enter_context(tc.tile_pool(name="ops", bufs=2, space="PSUM"))

    # persistent sbuf state
    xt = const.tile([128, NB, NCH, 512], BF16)    # x chunks: [c-part, blk, chunk, t]
    w_qk = const.tile([128, NCH, 128], BF16)      # [Wq|Wk] per c-chunk
    w_v = const.tile([128, NCH, 64], BF16)        # Wv per c-chunk
    qT_sb = const.tile([64, T], BF16)
    kT_sb = const.tile([64, T], BF16)
    v_sb = const.tile([128, T // 128, H + 1], BF16)  # v natural + ones col
    neg_mask_f = const.tile([128, 128], F32)     # 0 where t>=s, -1e9 below diag
    wu = const.tile([128, 16], BF16)             # warmup zeros

    # --- input DMAs first so transfers start immediately.
    nc.sync.dma_start(w_qk, wqk)
    for p in range(4):
        nc.sync.dma_start(xt[:, 0, 2 * p : 2 * p + 2, :],
                          xp[:, 0, 2 * p : 2 * p + 2, :])
    nc.sync.dma_start(w_v, wv)
    nc.sync.dma_start(xt[:, 1, 0:4, :], xp[:, 1, 0:4, :])
    nc.sync.dma_start(xt[:, 1, 4:6, :], xp[:, 1, 4:6, :])
    nc.sync.dma_start(xt[:, 1, 6:8, :], xp[:, 1, 6:8, :])
    nc.sync.dma_start(xt[:, 2], xp[:, 2])
    nc.sync.dma_start(xt[:, 3], xp[:, 3])

    # --- PE p-state ramp management (see header)
    nc.gpsimd.memset(wu, 0.0)
    wu_t = qk_ps.tile([128, 512], F32, tag="qk")
    nc.tensor.matmul(wu_t[0:16, 0:16], wu, wu, start=True, stop=True,
                     skip_group_check=True)
    nc.tensor.matmul(wu_t[0:16, 0:16], w_qk[:, 0, 0:16], w_qk[:, 0, 0:16],
                     start=True, stop=True, skip_group_check=True)
    for _ in range(2):
        nc.tensor.matmul(wu_t[0:16, 0:16], xt[:, 0, 0, 0:16],
                         xt[:, 0, 0, 0:16],
                         start=True, stop=True, skip_group_check=True)
    # preload the ACT exp table during the DMA wait
    junk = const.tile([128, 16], BF16)
    nc.scalar.activation(junk, wu, func=EXP, scale=0.125)

    # --- constants
    nc.gpsimd.memset(neg_mask_f, 0.0)
    nc.gpsimd.affine_select(
        out=neg_mask_f, in_=neg_mask_f, compare_op=mybir.AluOpType.is_ge,
        fill=NEG, base=0, pattern=[[1, 128]], channel_multiplier=-1,
    )
    ones_f = const.tile([128, 64], F32)
    nc.vector.memset(ones_f, 1.0)
    for j in range(T // 128):
        nc.vector.tensor_copy(v_sb[:, j, H : H + 1], ones_f[:, 0:1])

    # persistent v psum: 4 sequentially-closed accumulation groups in one
    # bank; the unused upper half of the bank hosts the last block's final
    # diag score so its chain is free of all st-slot dependencies
    v_nat = v_ps.tile([128, 512], F32)

    def qk_steps(b):
        """q/k projection of block b as single-instruction emission steps."""
        qk_t = qk_ps.tile([128, 512], F32, tag="qk")
        steps = []
        for j in range(NCH):
            steps.append(lambda j=j: nc.tensor.matmul(
                qk_t, w_qk[:, j, :], xt[:, b, j, :],
                start=(j == 0), stop=(j == NCH - 1), skip_group_check=True))

        blk = slice(512 * b, 512 * (b + 1))
        if b == 0:
            # split copies so the first diagonal scores (emitted in
            # [r2, r3, r0, r1] order) can start after the first halves
            def copies():
                nc.vector.tensor_copy(qT_sb[:, 256:512], qk_t[0:64, 256:512])
                nc.vector.tensor_copy(kT_sb[:, 256:512], qk_t[64:128, 256:512])
                nc.vector.tensor_copy(qT_sb[:, 0:256], qk_t[0:64, 0:256])
                nc.vector.tensor_copy(kT_sb[:, 0:256], qk_t[64:128, 0:256])
        else:
            # q first: block b's pair scores need only the NEW qT block
            # (their kT slices come from earlier blocks); kT block b is
            # needed later, by the diagonal units only.  High priority:
            # these gate the next block's scores AND (via the qk psum
            # slot WAR) the next projection -- the whole block-boundary
            # critical chain.
            def copies():
                nc.vector.tensor_copy(qT_sb[:, blk], qk_t[0:64, :])
                nc.vector.tensor_copy(kT_sb[:, blk], qk_t[64:128, :])
        steps.append(copies)
        return steps

    def v_mm_step(b, r):
        def mm():
            for j in range(NCH):
                nc.tensor.matmul(
                    v_nat[:, 64 * r : 64 * (r + 1)],
                    xt[:, b, j, 128 * r : 128 * (r + 1)], w_v[:, j, :],
                    start=(j == 0), stop=(j == NCH - 1), skip_group_check=True)
        return mm

    def v_cp_step(b, r):
        def cp():
            nc.vector.tensor_copy(v_sb[:, 4 * b + r, 0:H],
                                  v_nat[:, 64 * r : 64 * (r + 1)])
        return cp

    def v_steps(b):
        """v projection of block b (natural layout, x chunk stationary):
        att(b) PE filler, complete before att(b)'s first diagonal PV."""
        steps = []
        for r in range(4):
            steps.append(v_mm_step(b, r))
            steps.append(v_cp_step(b, r))
        return steps

    def units_for(b):
        """Emission units: off-diagonal chunk pairs, then diag singles."""
        if b == 0:
            return [("diag", 2), ("diag", 3), ("diag", 0), ("diag", 1)]
        us = [("pair", (2 * p, 2 * p + 1)) for p in range(2 * b)]
        us += [("diag", r) for r in range(4)]
        return us

    def chunks_of(b, u):
        """(chunk j, 512-slice getter, min group) per chunk of a unit."""
        kind, d = u
        if kind == "pair":
            return [(d[0], 0, 0), (d[1], 1, 0)]
        return [(4 * b + d, 0, d)]

    def pv_plan(b, units, extra=0):
        """Per-group contributor counts for this block's PV accumulation."""
        cnt = [extra] * 4
        for u in units:
            for (_, _, gmin) in chunks_of(b, u):
                for g in range(gmin, 4):
                    cnt[g] += 1
        return cnt

    def att_emit(b, vfill, qkfill, pre, last_ctx=None):
        """Emit attention for block b.  `vfill` (v of b, complete before the
        first diagonal PV) and `qkfill` (q/k of b+1) are spread between PV
        steps; `pre` is the previous block's deferred normalize (PE-free)."""
        last = (b == NB - 1)
        if last:
            # j=2..7 ran early at the end of att(2); diagonals last, in r
            # order, so the four output groups close staggered and the
            # final exp (width 128) is the smallest
            units = [("pair", (0, 1)), ("pair", (8, 9)), ("pair", (10, 11)),
                     ("diag", 0), ("diag", 1), ("diag", 2), ("diag", 3)]
            ot, seen, cnt = last_ctx
        else:
            units = units_for(b)
            ot = o_ps.tile([128, 4, H + 1], F32, tag="o", name="ot")
            seen = [0] * 4
            cnt = pv_plan(b, units)
        nu = len(units)
        pvs = [None] * nu
        # Diag score tiles are shared pairwise, interleaved by r-parity
        # (r0+r2 in one tile, r1+r3 in the other): PSUM WAR deps are
        # tile-granular, so a same-tile successor score must wait for its
        # partner's exp -- parity interleaving hides that wait behind the
        # intervening diag's exp.  In the last block the odd-parity pair
        # lives in the (now idle) qk bank, freeing both st slots.
        dslot = {0: None, 1: None}

        def diag_st(r, width):
            if last and r == 3:
                # the v bank's idle upper half: no st-slot WAR at all, so
                # the kernel's final score/exp chain starts early
                return v_nat[:, 384:512]
            p = r % 2
            held = dslot[p]
            if held is None:
                if last and p == 1:
                    stq = qk_ps.tile([128, 512], F32, tag="qk", name="stq")
                    dslot[p] = ("q", stq)
                    return stq[:, 0:width]
                stw = st_ps.tile([128, 1024], F32, tag="st", name="std")
                dslot[p] = ("s", stw)
                return stw[:, 0:width]
            kind, t = held
            dslot[p] = None
            if kind == "q":
                return t[:, 512 - width : 512]
            return t[:, 512 : 512 + width]

        def score_unit(i):
            kind, d = units[i]
            qblk = qT_sb[:, 512 * b : 512 * (b + 1)]
            if kind == "pair":
                j0, j1 = d
                st = st_ps.tile([128, 1024], F32, tag="st", name="stp")
                ptp = pt_pool.tile([128, 1024], BF16, tag="ptp", bufs=6,
                                   name="ptp")
                nc.tensor.matmul(st[:, 0:512], kT_sb[:, 128 * j0 : 128 * (j0 + 1)],
                                 qblk, start=True, stop=True,
                                 skip_group_check=True)
                nc.tensor.matmul(st[:, 512:1024],
                                 kT_sb[:, 128 * j1 : 128 * (j1 + 1)],
                                 qblk, start=True, stop=True,
                                 skip_group_check=True)
                nc.scalar.activation(ptp, st, func=EXP, scale=0.125)
                pvs[i] = [(j0, ptp[:, 0:512], 0), (j1, ptp[:, 512:1024], 0)]
            else:
                # diag: exp the raw scores, then zero the below-diagonal
                # 128x128 region of PT on the idle gpsimd engine -- keeps
                # the DVE mask-add off the score->exp critical path.  The
                # g>r PVs read only exp-written columns, so they are
                # gated on the exp alone (byte-range deps).  The last
                # block's final diag keeps the pre-exp DVE mask instead:
                # its group close is the kernel tail, and mask-then-exp
                # is the shorter post-score chain there.
                r = d
                j = 4 * b + r
                coff = 128 * r
                width = 512 - coff
                st = diag_st(r, width)
                pt = pt_pool.tile([128, 512], BF16, tag="ptd", bufs=4,
                                  name="ptd")
                nc.tensor.matmul(st[:, 0:width], kT_sb[:, 128 * j : 128 * (j + 1)],
                                 qT_sb[:, 512 * b + coff : 512 * (b + 1)],
                                 start=True, stop=True, skip_group_check=True)
                if last:
                    # last block: pre-exp DVE mask (DVE is idle here, the
                    # masks complete early) so each group-close chain is
                    # exp -> PV -> normalize with no gpsimd select hop
                    nc.vector.tensor_add(st[:, 0:128], st[:, 0:128],
                                         neg_mask_f)
                    nc.scalar.activation(pt[:, coff:512], st[:, 0:width],
                                         func=EXP, scale=0.125)
                else:
                    nc.scalar.activation(pt[:, coff:512], st[:, 0:width],
                                         func=EXP, scale=0.125)
                    nc.gpsimd.affine_select(
                        out=pt[:, coff : coff + 128],
                        in_=pt[:, coff : coff + 128],
                        compare_op=mybir.AluOpType.is_ge,
                        fill=0.0, base=0, pattern=[[1, 128]],
                        channel_multiplier=-1,
                    )
                pvs[i] = [(j, pt, r)]

        def close_group(g):
            """Normalize + store output rows [512b+128g : 512b+128(g+1)]."""
            rc = rc_pool.tile([128, 1], F32)
            nc.vector.reciprocal(rc, ot[:, g, H : H + 1])
            fin = fin_pool.tile([128, H], F32, tag="fin", bufs=6, name="fin")
            nc.vector.tensor_scalar_mul(fin, ot[:, g, 0:H], rc)
            if last:
                # spread the endgame stores across DGE paths: gpsimd's
                # SWDGE generation runs on the idle Q7, off the shared
                # HWDGE generator; the final store issues from the idle
                # ACT queue
                q = {0: nc.gpsimd, 1: nc.sync, 2: nc.sync,
                     3: nc.scalar}[g]
            else:
                q = nc.sync
            q.dma_start(outN[b, g], fin)

        # PSUM allows only ONE open accumulation group per bank: a second
        # start=True wipes the previous start's (unstopped) data.  So the
        # very first PV into the ot bank carries start=True (zeroing the
        # whole bank), every other PV accumulates, and the block's final
        # PV carries the stop.
        def pv(i):
            for (j, pt512, gmin) in pvs[i]:
                for g in range(gmin, 4):
                    nc.tensor.matmul(ot[:, g, :],
                                     pt512[:, 128 * g : 128 * (g + 1)],
                                     v_sb[:, j, :],
                                     start=(sum(seen) == 0),
                                     stop=(sum(seen) == sum(cnt) - 1),
                                     skip_group_check=True)
                    seen[g] += 1
                    if last and seen[g] == cnt[g]:
                        close_group(g)

        filler = list(vfill) + list(qkfill)
        nvf = len(vfill)
        fi = 0

        # score emission pointer: diag units chain their dslot partner so
        # consecutive diagonal exps queue back-to-back on ACT
        si = 0

        def emit_scores(limit):
            nonlocal si
            while si < nu and si < limit:
                is_diag = units[si][0] == "diag"
                score_unit(si)
                si += 1
                if is_diag:
                    # the diag run fits in two parity-interleaved tiles:
                    # emit it whole so the exps queue back-to-back on ACT
                    while si < nu and units[si][0] == "diag":
                        score_unit(si)
                        si += 1

        # block 0 is all-diagonal: with pairwise diag slots the whole
        # lookahead is 4 units, so emit every score upfront
        la = 4 if b == 0 else 2
        emit_scores(la)
        # the last block's deferred-normalize predecessor must run before
        # its very first PV (psum slot recycling); earlier blocks' can wait
        # until the diagonal drain
        prestate = [pre]
        if last and pre is not None:
            pre()
            prestate[0] = None

        def fillers(i):
            nonlocal fi
            if units[i][0] == "diag":   # diagonal PV: v(b) must be in place
                if prestate[0] is not None:
                    prestate[0]()
                    prestate[0] = None
                while fi < nvf:
                    filler[fi]()
                    fi += 1
            if b == 0:
                den = 1     # qk(1) is DMA-gated; b0's PVs wait v-copies
                            # anyway, so drain every filler immediately
            elif b == 1:
                den = 3
            elif b == 2:
                den = 4
            else:
                # last block: v(3)'s copies must be emitted before the
                # diag-run scores -- d15's score shares the v psum tile,
                # and deps are tile-granular
                den = 1
            want = min(len(filler), (len(filler) * (i + 1)) // den)
            while fi < want:
                filler[fi]()
                fi += 1

        for i in range(nu):
            fillers(i)
            emit_scores(i + 1 + la)
            pv(i)
        while fi < len(filler):
            filler[fi]()
            fi += 1

        if last:
            return None

        def normalize():
            for g in range(4):
                close_group(g)
        return normalize

    # ---- top-level emission
    for s in qk_steps(0):
        s()
    v0 = v_steps(0)
    # v(0) matmul steps fill the PE gap while the block-0 q/k copies run;
    # the psum->sbuf copies stay as att(0) fillers (they must not delay
    # the boundary copies on DVE)
    for s in [v0[0], v0[2], v0[4], v0[6]]:
        s()
    pre = att_emit(0, [v0[1], v0[3], v0[5], v0[7]], qk_steps(1), None)
    pre = att_emit(1, v_steps(1), qk_steps(2), pre)
    vf2 = v_steps(2)
    qf3 = qk_steps(3)
    pre = att_emit(2, vf2, qf3, pre)
    # early block-3 units (j=2..7 as three pairs): their scores+exps fill
    # the ACT seam at the att2/att3 boundary and their PVs open the
    # block-3 output groups
    units3 = [("pair", (0, 1)), ("pair", (8, 9)), ("pair", (10, 11)),
              ("diag", 0), ("diag", 1), ("diag", 2), ("diag", 3)]
    cnt3 = pv_plan(3, units3, extra=6)
    seen3 = [0] * 4
    ot3 = o_ps.tile([128, 4, H + 1], F32, tag="o", name="ot3")
    for pi, (j0, j1) in enumerate([(2, 3), (4, 5), (6, 7)]):
        st = st_ps.tile([128, 1024], F32, tag="st", name="ste")
        ptp = pt_pool.tile([128, 1024], BF16, tag="ptp", bufs=6, name="ptpe")
        nc.tensor.matmul(st[:, 0:512], kT_sb[:, 128 * j0 : 128 * (j0 + 1)],
                         qT_sb[:, 1536:2048], start=True, stop=True,
                         skip_group_check=True)
        nc.tensor.matmul(st[:, 512:1024], kT_sb[:, 128 * j1 : 128 * (j1 + 1)],
                         qT_sb[:, 1536:2048], start=True, stop=True,
                         skip_group_check=True)
        nc.scalar.activation(ptp, st, func=EXP, scale=0.125)
        for (j, half) in ((j0, 0), (j1, 1)):
            for g in range(4):
                nc.tensor.matmul(ot3[:, g, :],
                                 ptp[:, 512 * half + 128 * g :
                                        512 * half + 128 * (g + 1)],
                                 v_sb[:, j, :],
                                 start=(sum(seen3) == 0), stop=False,
                                 skip_group_check=True)
                seen3[g] += 1
    att_emit(3, v_steps(3), [], pre, last_ctx=(ot3, seen3, cnt3))


_NC = None


def _get_nc():
    global _NC
    if _NC is None:
        _NC = build_bass()
    return _NC


def kernel(x, Wq, Wk, Wv):
    nc = _get_nc()
    wqk_h = np.concatenate([Wq, Wk], axis=1)               # [1024, 128]
    wqk_h = np.ascontiguousarray(
        wqk_h.reshape(8, 128, 128).transpose(1, 0, 2)      # [128, 8, 128]
    ).astype(ml_dtypes.bfloat16)
    wv_h = np.ascontiguousarray(
        Wv.reshape(8, 128, 64).transpose(1, 0, 2)          # [128, 8, 64]
    ).astype(ml_dtypes.bfloat16)
    in_maps = []
    for b in range(8):
        xT = np.ascontiguousarray(x[b].T)                  # [1024, 2048]
        xpk = np.ascontiguousarray(
            xT.reshape(8, 128, 4, 512).transpose(1, 2, 0, 3)  # [128, 4, 8, 512]
        ).astype(ml_dtypes.bfloat16)
        in_maps.append({"xp": xpk, "wqk": wqk_h, "wv": wv_h})
    res = bass_utils.run_bass_kernel_spmd(nc, in_maps, core_ids=list(range(8)))
    out = np.stack([np.asarray(res.results[b]["outN"],
                               dtype=np.float32).reshape(T, H)
                    for b in range(8)])
    return out
